# revision 10
# baseline (speedup 1.0000x reference)
"""Trainium2 Bass kernel for nn_ColorTransform: per-pixel degree-3 polynomial
color transform  y[b,c,h,w] = bias[c] + sum_f weight[f,c] * mono_f(x[b,:,h,w]).

Pure data parallel over batch across 8 cores (identical SPMD program).

Algorithm: represent the 3->19->3 degree-3 polynomial map as

    y_c = sum_{i<R} cq[i,c] * L_i^3 + cs[i,c] * L_i^2,   L_i = a_i.x + b_i

with R=7 forms solved per-(weight,bias) on the host by Gauss-Newton.  The
forms are cascade-rounded so (a_i, b_i) are exactly representable in f16
(the M1 matmul dtype); coefficients re-solved after each freeze.  Fallback
ladder R=7 -> 8 -> 9 -> 10 (R=10 always solvable by plain lstsq for any
target, quantization handled by the same cascade machinery).

On-chip, pixels are packed GROUPS-per-chunk x pixel-columns on the partition
dim (R form-rows per group, NG = BPC*GPB groups):
  PE  M1  -> P1 = wm1^T @ X  (block-diag forms)      [R*NG, NCMP] PSUM
  ACT     -> S = Square(P1)                          [R*NG] SBUF f32r
  DVE     -> Q = S * P1                              [R*NG] SBUF f32r
  PE  M2  -> P2 = w2q^T Q + w2s^T S  (PSUM accum)    [3*NG (+54 odd unit)]
  ACT/DVE -> pair copy-out (2 units share one PSUM tile: one copy per pair)
  DMA out f16 (host converts back to f32)
Full chunks: GPB=9 groups/batch x ND=4096 cols; tail chunk GPB=8 x 512.
"""
import numpy as np
from itertools import product as _product
from math import factorial as _factorial

import concourse.bass as bass
import concourse.tile as tile
from concourse import bacc, mybir
from concourse.bass_utils import run_bass_kernel_spmd

# ---------------------------------------------------------------- constants
B, C, H, W = 16, 3, 512, 512
HW = H * W
NCORES = 8
BPC = B // NCORES          # batches per core = 2

R = 7                      # affine forms per group (adaptive solve)
GPB = 9                    # groups per batch (full chunks)
ND = 4096                  # pixel columns per full chunk
NCMP = int(__import__("os").environ.get("K_NCMP", "512"))
SPLIT = ND // NCMP         # 4 units per chunk
FULL_CHUNKS = 7            # 9*4096*7 = 258048 px/plane
TAIL_GPB = 8               # tail: 8 groups/batch x 512 cols = 4096 px/plane
TAIL_ND = 512

import os
ACT_COPY_FRAC = float(os.environ.get("K_ACT_FRAC", "0.66"))
PS_BUFS = int(os.environ.get("K_PS_BUFS", "3"))
P2_BUFS = int(os.environ.get("K_P2_BUFS", "2"))
D2 = int(os.environ.get("K_D2", "1"))   # stage1 -> stage2 emission lag (units)
D3 = int(os.environ.get("K_D3", "2"))   # stage2 -> stage3 emission lag (units)
D4 = int(os.environ.get("K_D4", "1"))   # stage3 -> stage4 emission lag (pairs)
SQ_BUFS = int(os.environ.get("K_SQ_BUFS", "4"))
ABLATE = os.environ.get("K_ABLATE", "")
COPY_MODE = os.environ.get("K_COPY_MODE", "split")
UNROLL_REPS = int(os.environ.get("K_UNROLL_REPS", "4"))

assert GPB * ND * FULL_CHUNKS + TAIL_GPB * TAIL_ND == HW

# ------------------------------------------------------------ monomial alg
ALL_MONO = [(0,0,0),
    (1,0,0),(0,1,0),(0,0,1),
    (2,0,0),(1,1,0),(1,0,1),(0,2,0),(0,1,1),(0,0,2),
    (3,0,0),(2,1,0),(2,0,1),(1,2,0),(1,1,1),(1,0,2),(0,3,0),(0,2,1),(0,1,2),(0,0,3)]
MIDX = {m: i for i, m in enumerate(ALL_MONO)}

# multiply-by-x_k operators in the 20-dim monomial basis (degree<=2 source)
_XK = []
for k in range(3):
    m = np.zeros((20, 20))
    for mono, i in MIDX.items():
        if sum(mono) <= 2:
            up = list(mono); up[k] += 1
            m[MIDX[tuple(up)], i] = 1.0
    _XK.append(m)


def _expand_pow(a, b, power):
    out = np.zeros(20)
    for ks in _product(range(power + 1), repeat=4):
        if sum(ks) != power:
            continue
        k0, k1, k2, kb = ks
        mult = _factorial(power) / (_factorial(k0)*_factorial(k1)*_factorial(k2)*_factorial(kb))
        out[MIDX[(k0, k1, k2)]] += mult * a[0]**k0 * a[1]**k1 * a[2]**k2 * b**kb
    return out


def _basis(a, b, R_):
    e3 = np.stack([_expand_pow(a[i], b[i], 3) for i in range(R_)], axis=1)
    e2 = np.stack([_expand_pow(a[i], b[i], 2) for i in range(R_)], axis=1)
    e1 = np.stack([_expand_pow(a[i], b[i], 1) for i in range(R_)], axis=1)
    return e3, e2, e1


def _unpack(theta, R_):
    a = theta[:3*R_].reshape(R_, 3)
    b = theta[3*R_:4*R_]
    cq = theta[4*R_:7*R_].reshape(R_, 3)
    cs = theta[7*R_:10*R_].reshape(R_, 3)
    return a, b, cq, cs


def _pack(a, b, cq, cs):
    return np.concatenate([a.ravel(), b, cq.ravel(), cs.ravel()])


def _residual_jac(theta, R_, T, want_jac=True):
    a, b, cq, cs = _unpack(theta, R_)
    e3, e2, e1 = _basis(a, b, R_)
    M = e3 @ cq + e2 @ cs
    r = (M - T).ravel()            # row-major: 20 rows x 3 channels
    if not want_jac:
        return r, None
    J = np.zeros((60, 10*R_))
    for i in range(R_):
        # d/da_i[k]: 3*(Xk @ e2_i) * cq[i,c] + 2*(Xk @ e1_i) * cs[i,c]
        for k in range(3):
            col = 3*_XK[k] @ e2[:, i]
            col2 = 2*_XK[k] @ e1[:, i]
            for c in range(3):
                J[c::3, 3*i+k] += col * cq[i, c] + col2 * cs[i, c]
        # d/db_i: 3*e2_i*cq + 2*e1_i*cs
        for c in range(3):
            J[c::3, 3*R_+i] += 3*e2[:, i]*cq[i, c] + 2*e1[:, i]*cs[i, c]
        for c in range(3):
            J[c::3, 4*R_+3*i+c] = e3[:, i]
            J[c::3, 7*R_+3*i+c] = e2[:, i]
    return r, J


def _lm_solve(theta0, R_, T, free_mask, max_iter=200, tol=1e-13):
    """Levenberg-Marquardt over theta[free_mask]."""
    theta = theta0.copy()
    lam = 1e-3
    r, J = _residual_jac(theta, R_, T)
    cost = r @ r
    for _ in range(max_iter):
        Jf = J[:, free_mask]
        g = Jf.T @ r
        A = Jf.T @ Jf
        ok = False
        for _try in range(25):
            try:
                dx = np.linalg.solve(A + lam*np.diag(np.maximum(np.diag(A), 1e-10)), -g)
            except np.linalg.LinAlgError:
                lam *= 10; continue
            th_new = theta.copy()
            th_new[free_mask] = theta[free_mask] + dx
            r_new, _ = _residual_jac(th_new, R_, T, want_jac=False)
            c_new = r_new @ r_new
            if c_new < cost:
                theta, cost = th_new, c_new
                lam = max(lam*0.3, 1e-12)
                ok = True
                break
            lam *= 10
        if not ok:
            break
        r, J = _residual_jac(theta, R_, T)
        if cost < tol**2:
            break
    return theta, np.sqrt(cost)


def _coeff_lstsq(a, b, R_, T):
    e3, e2, _ = _basis(a, b, R_)
    A = np.concatenate([e3, e2], axis=1)
    Cf, *_ = np.linalg.lstsq(A, T, rcond=None)
    resid = np.linalg.norm(A @ Cf - T)
    return Cf[:R_], Cf[R_:], resid


def _normalize(theta, R_):
    a, b, cq, cs = _unpack(theta, R_)
    for i in range(R_):
        hi = b[i] + np.clip(a[i], 0, None).sum()
        lo = b[i] + np.clip(a[i], None, 0).sum()
        t = 1.0 / max(abs(hi), abs(lo), 1e-6)
        a[i] *= t; b[i] *= t
        cq[i] /= t**3; cs[i] /= t**2
    return _pack(a, b, cq, cs)


def _cascade_round(theta, R_, T):
    theta = theta.copy()
    frozen = np.zeros(10*R_, dtype=bool)
    for i in range(R_):
        idx = np.r_[3*i:3*i+3, 3*R_+i]
        theta[idx] = np.float64(np.float16(theta[idx]))
        frozen[idx] = True
        if i < R_ - 1:
            theta, r = _lm_solve(theta, R_, T, ~frozen, max_iter=60)
        else:
            a, b, cq, cs = _unpack(theta, R_)
            cq, cs, r = _coeff_lstsq(a, b, R_, T)
            theta = _pack(a, b, cq, cs)
    return theta, r


def _target(weight, bias):
    T = np.zeros((20, 3))
    T[0] = np.asarray(bias, np.float64)
    T[1:] = np.asarray(weight, np.float64)
    return T


def solve_forms(weight, bias, R_=R, n_init=8, seed=0):
    """-> (a [R,3] f16-exact, b [R] f16-exact, cq [R,3], cs [R,3], resid)."""
    T = _target(weight, bias)
    rg = np.random.default_rng(seed)
    best = None
    for trial in range(n_init):
        a0 = rg.normal(size=(R_, 3))
        a0 /= np.linalg.norm(a0, axis=1, keepdims=True)
        b0 = rg.normal(size=R_) * 0.5
        cq0, cs0, _ = _coeff_lstsq(a0, b0, R_, T)
        th0 = _pack(a0, b0, cq0, cs0)
        theta, r = _lm_solve(th0, R_, T, np.ones(10*R_, dtype=bool))
        if r > 1e-9:
            continue
        theta = _normalize(theta, R_)
        thq, rq = _cascade_round(theta, R_, T)
        a, b, cq, cs = _unpack(thq, R_)
        coefmag = max(np.abs(cq).max(), np.abs(cs).max())
        score = rq * (1 + 0.001*coefmag)
        if best is None or score < best[0]:
            best = (score, thq, rq)
        if rq < 5e-5 and coefmag < 50:
            break
    if best is None:
        return None
    a, b, cq, cs = _unpack(best[1], R_)
    return a, b, cq, cs, best[2]


# ---------------------------------------------------------- weight layouts
# X rows:  0 = ones; 1 + b*(3*gpb) + v*gpb + g
# P1 rows: b*(R*gpb) + i*gpb + g
# P2/O rows: b*(3*gpb) + c*gpb + g  (+ 3*gpb*BPC for odd units of a pair)

def _lhs1(av, bv, gpb):
    kx = 3*gpb*BPC + 1
    m = np.zeros((kx, R*gpb*BPC), np.float32)
    for b in range(BPC):
        for g in range(gpb):
            for i in range(R):
                col = b*R*gpb + i*gpb + g
                m[0, col] = bv[i]
                for v in range(C):
                    m[1 + b*3*gpb + v*gpb + g, col] = av[i, v]
    return m.astype(np.float16)


def _lhs2(coeff, gpb, parity=None):
    """M2 stationary: [R*gpb*BPC, OW] (parity=None) or zero-padded
    [R*gpb*BPC, 64+OW] with the coefficient block at column 64*parity."""
    ow = 3*gpb*BPC
    base = 0 if parity is None else 64*parity
    cols = ow if parity is None else 64 + ow if parity == 1 else 64 + ow
    if parity is not None:
        cols = 64 + ow
    m = np.zeros((R*gpb*BPC, cols if parity is not None else ow), np.float32)
    for b in range(BPC):
        for g in range(gpb):
            for i in range(R):
                for c in range(C):
                    m[b*R*gpb + i*gpb + g, base + b*3*gpb + c*gpb + g] = coeff[i, c]
    return m


# ---------------------------------------------------------------- bass build
_NC_CACHE = {}


def build_nc(reps=1, chunks=None):
    key = (reps, chunks)
    if key in _NC_CACHE:
        return _NC_CACHE[key]
    f32, f16, f32r = mybir.dt.float32, mybir.dt.float16, mybir.dt.float32r
    nc = bacc.Bacc("TRN2", target_bir_lowering=False, debug=False, num_devices=NCORES)

    KX = 3*GPB*BPC + 1          # 55
    RW = R*GPB*BPC              # 126
    OW = 3*GPB*BPC              # 54
    KXt = 3*TAIL_GPB*BPC + 1    # 49
    RWt = R*TAIL_GPB*BPC        # 112
    OWt = 3*TAIL_GPB*BPC       # 48

    xs = nc.dram_tensor("xs", [BPC, C, HW], f16, kind="ExternalInput")
    wm1 = nc.dram_tensor("wm1", [KX, RW], f16, kind="ExternalInput")
    w2qe = nc.dram_tensor("w2qe", [RW, 64 + OW], f32, kind="ExternalInput")
    w2se = nc.dram_tensor("w2se", [RW, 64 + OW], f32, kind="ExternalInput")
    w2qo = nc.dram_tensor("w2qo", [RW, 64 + OW], f32, kind="ExternalInput")
    w2so = nc.dram_tensor("w2so", [RW, 64 + OW], f32, kind="ExternalInput")
    wm1t = nc.dram_tensor("wm1t", [KXt, RWt], f16, kind="ExternalInput")
    w2qt = nc.dram_tensor("w2qt", [RWt, OWt], f32, kind="ExternalInput")
    w2st = nc.dram_tensor("w2st", [RWt, OWt], f32, kind="ExternalInput")
    yp = nc.dram_tensor("yp", [2, OW, FULL_CHUNKS if chunks is None else max(chunks, 1), ND//2],
                        f16, kind="ExternalOutput")
    yt = nc.dram_tensor("yt", [OWt, TAIL_ND], f16, kind="ExternalOutput")

    asp = (int(NCMP * ACT_COPY_FRAC) // 2) * 2

    with tile.TileContext(nc) as tc:
        with (
            tc.tile_pool(name="wpool", bufs=1) as wpool,
            tc.tile_pool(name="xpool", bufs=int(os.environ.get("K_X_BUFS", "4"))) as xpool,
            tc.tile_pool(name="spool", bufs=SQ_BUFS) as spool,
            tc.tile_pool(name="qpool", bufs=SQ_BUFS) as qpool,
            tc.tile_pool(name="opool", bufs=int(os.environ.get("K_O_BUFS", "3"))) as opool,
            tc.tile_pool(name="p1pool", bufs=PS_BUFS, space="PSUM") as p1pool,
            tc.tile_pool(name="p2pool", bufs=P2_BUFS, space="PSUM") as p2pool,
        ):
            def load_w(name, dram, shape, dt_, round_to=None):
                t = wpool.tile(shape, dt_, tag=name)
                nc.sync.dma_start(t[:], dram[:])
                if round_to is None:
                    return t
                tr = wpool.tile(shape, round_to, tag=name + "r")
                nc.vector.tensor_copy(tr[:], t[:])
                return tr

            wm1_sb = load_w("wm1", wm1, [KX, RW], f16)
            w2qe_r = load_w("w2qe", w2qe, [RW, 64 + OW], f32, f32r)
            w2se_r = load_w("w2se", w2se, [RW, 64 + OW], f32, f32r)
            w2qo_r = load_w("w2qo", w2qo, [RW, 64 + OW], f32, f32r)
            w2so_r = load_w("w2so", w2so, [RW, 64 + OW], f32, f32r)
            wm1t_sb = load_w("wm1t", wm1t, [KXt, RWt], f16)
            w2qt_r = load_w("w2qt", w2qt, [RWt, OWt], f32, f32r)
            w2st_r = load_w("w2st", w2st, [RWt, OWt], f32, f32r)

            for _ in range(int(os.environ.get("K_X_BUFS", "4"))):
                xt0 = xpool.tile([KX, ND], f16, tag="X")
                nc.gpsimd.memset(xt0[0:1, :], 1.0)
            if ABLATE == "nosq":
                for _ in range(SQ_BUFS):
                    s0 = spool.tile([RW, 2*NCMP], f32r, tag="S")
                    nc.gpsimd.memset(s0[:], 0.25)
                    q0 = qpool.tile([RW, 2*NCMP], f32r, tag="Q")
                    nc.gpsimd.memset(q0[:], 0.125)

            # ---------------- stages -------------------------------------
            def stage0(ck, lo, gpb, nd, st):
                """Prefetch chunk ck's X tile (DMA-in)."""
                kx = 3*gpb*BPC + 1
                xt = xpool.tile([kx, nd], f16, tag="X",
                                name=f"xt{'T' if gpb == TAIL_GPB else ''}")
                if gpb == TAIL_GPB:
                    nc.gpsimd.memset(xt[0:1, :], 1.0)
                nc.sync.dma_start(
                    xt[1:1+2*3*gpb],
                    xs[:, :, lo:lo+gpb*nd].rearrange(
                        "b v (g n) -> (b v) g n", n=nd))
                st[ck] = xt

            # A "pair" = 2 consecutive NCMP-column units sharing one P1 tile
            # (M1 writes halves; square/cube run once over both halves).
            # A "quad" = 2 consecutive pairs sharing one P2 tile
            # (even/odd unit -> partition base 0/64, pair parity -> col half);
            # one copy-out per quad.
            def stage1(pr, st):
                """M1 for both halves of pair -> P1 [rw, 2*ncmp]."""
                ck, lo, gpb, nd, ncmp, p, wset, nun = pr
                rw = R*gpb*BPC
                xt = st[ck]
                p1 = p1pool.tile([rw, 2*NCMP], f32, tag="P1")
                for h in range(nun):
                    cl = (2*p + h)*ncmp
                    nc.tensor.matmul(p1[:, h*ncmp:(h+1)*ncmp], wset[0][:],
                                     xt[:, cl:cl+ncmp], start=True, stop=True)
                return p1

            def stage2(pr, p1):
                ck, lo, gpb, nd, ncmp, p, wset, nun = pr
                rw = R*gpb*BPC
                w = nun*ncmp
                s = spool.tile([rw, 2*NCMP], f32r, tag="S")
                if ABLATE != "nosq":
                    nc.scalar.square(s[:, :w], p1[:, :w])
                qq = qpool.tile([rw, 2*NCMP], f32r, tag="Q")
                if ABLATE != "nosq":
                    nc.vector.tensor_mul(qq[:, :w], s[:, :w], p1[:, :w])
                return s, qq

            def stage3(pr, s, qq, st):
                """M2 into row-paired PSUM [64+OW, NCMP] via zero-padded
                block lhsT (both units accumulate, dst base 0)."""
                ck, lo, gpb, nd, ncmp, p, wset, nun = pr
                ow = 3*gpb*BPC
                tail = gpb == TAIL_GPB
                rows = ow if tail else 64 + ow
                p2 = p2pool.tile([rows, NCMP], f32, tag="P2",
                                 name="p2T" if tail else "p2")
                for h in range(nun):
                    cb = h*ncmp
                    qw = wset[1] if tail else wset[1][h]
                    sw = wset[2] if tail else wset[2][h]
                    nc.tensor.matmul(p2[:, :ncmp], qw[:], qq[:, cb:cb+ncmp],
                                     start=(h == 0), stop=False)
                    nc.tensor.matmul(p2[:, :ncmp], sw[:], s[:, cb:cb+ncmp],
                                     start=False, stop=(h == nun - 1))
                if tail:
                    st["o"] = opool.tile([ow, nd], f16, tag="O", name="oT")
                elif p == 0:
                    st["o"] = opool.tile([64 + ow, nd//2], f16, tag="O",
                                         name="o")
                return (pr, p2, st["o"])

            def stage4(pair):
                """Copy pair PSUM -> O (ACT/DVE col split); DMA at chunk end."""
                if ABLATE == "nocopy":
                    return
                pr, p2, o = pair
                ck, lo, gpb, nd, ncmp, p, wset, nun = pr
                ow = 3*gpb*BPC
                tail = gpb == TAIL_GPB
                if tail:
                    a = (int(ncmp*ACT_COPY_FRAC)//2)*2
                    nc.scalar.copy(o[:, :a], p2[:, :a])
                    nc.vector.tensor_copy(o[:, a:ncmp], p2[:, a:ncmp])
                    if ABLATE != "nodma":
                        nc.sync.dma_start(yt[:], o[:])
                    return
                cl = p*NCMP
                if COPY_MODE == "alt":
                    if p % 2 == 0:
                        nc.scalar.copy(o[:, cl:cl+NCMP], p2[:, :NCMP])
                    else:
                        nc.vector.tensor_copy(o[:, cl:cl+NCMP], p2[:, :NCMP])
                else:
                    nc.scalar.copy(o[:, cl:cl+asp], p2[:, :asp])
                    nc.vector.tensor_copy(o[:, cl+asp:cl+NCMP], p2[:, asp:NCMP])
                if (2*p + nun)*ncmp == nd and ABLATE != "nodma":
                    # device-order output: host inverse-permutes (free here)
                    for pp in range(2):
                        nc.sync.dma_start(yp[pp, :, ck, :],
                                          o[pp*64:pp*64+ow])

            def body():
                nfull = FULL_CHUNKS if chunks is None else chunks
                pairs = [(k, k*GPB*ND, GPB, ND, NCMP, p,
                          (wm1_sb, (w2qe_r, w2qo_r), (w2se_r, w2so_r)), 2)
                         for k in range(nfull) for p in range(SPLIT//2)]
                if chunks is None:
                    pairs.append((nfull, FULL_CHUNKS*GPB*ND, TAIL_GPB, TAIL_ND,
                                  TAIL_ND, 0, (wm1t_sb, w2qt_r, w2st_r), 1))
                chunk_info = {}
                for pr in pairs:
                    chunk_info.setdefault(pr[0], (pr[1], pr[2], pr[3]))
                st1, st3 = {}, {}
                q2, q3, q4 = [], [], []

                def pump(force=False):
                    if q2 and (force or len(q2) > D2 - 1):
                        pr, p1 = q2.pop(0)
                        s, qq = stage2(pr, p1)
                        q3.append((pr, s, qq))
                    if q3 and (force or len(q3) > D3 - 1):
                        pr, s, qq = q3.pop(0)
                        quad = stage3(pr, s, qq, st3)
                        if quad is not None:
                            q4.append(quad)
                    if q4 and (force or len(q4) > D4 - 1):
                        stage4(q4.pop(0))

                stage0(0, *chunk_info[0], st1)
                for pr in pairs:
                    ck, lo, gpb, nd, ncmp, p, wset, nun = pr
                    if p == 0 and ck + 1 in chunk_info:
                        stage0(ck + 1, *chunk_info[ck + 1], st1)
                    p1 = stage1(pr, st1)
                    q2.append((pr, p1))
                    pump()
                while q2 or q3 or q4:
                    pump(force=True)

            if reps == 1:
                for _u in range(int(os.environ.get("K_UNROLL", "1"))):
                    body()
            else:
                hint = (mybir.EngineType.PE, mybir.EngineType.Activation,
                        mybir.EngineType.DVE, mybir.EngineType.SP)
                u = UNROLL_REPS
                while u > 1 and reps % u:
                    u -= 1
                with tc.For_i(0, reps // u, 1, hint_engines=hint):
                    for _ in range(u):
                        body()

    nc.compile()
    _NC_CACHE[key] = nc
    return nc


# ------------------------------------------------------------- host wrapper
_SOLVE_CACHE = {}


def _eval_maxrel(a, b, cq, cs, weight, bias, n=60000):
    """Quick numpy check of the decomposition through the f16/f32 pipeline."""
    rng = np.random.default_rng(11)
    x = rng.uniform(0, 1, size=(n, 3))
    F = np.stack([x[:, 0]**m[0] * x[:, 1]**m[1] * x[:, 2]**m[2]
                  for m in ALL_MONO], axis=1)
    y_ref = F @ _target(weight, bias)
    L = x.astype(np.float16).astype(np.float32) @ a.T.astype(np.float32) \
        + b.astype(np.float32)
    S = L*L
    Q = S*L
    yy = (Q @ cq.astype(np.float32) + S @ cs.astype(np.float32))
    yy = yy.astype(np.float16).astype(np.float32)
    return np.abs(yy - y_ref).max() / max(np.abs(y_ref).max(), 1e-9)


def _solve_cached(weight, bias):
    key = (weight.tobytes(), bias.tobytes())
    if key in _SOLVE_CACHE:
        return _SOLVE_CACHE[key]
    best = None
    for seed in (0, 1, 2, 3):
        out = solve_forms(weight, bias, R_=R, seed=seed)
        if out is None:
            continue
        a, b, cq, cs, _resid = out
        err = _eval_maxrel(a, b, cq, cs, weight, bias)
        if best is None or err < best[0]:
            best = (err, (a, b, cq, cs))
        if err < 5e-3:
            break
    if best is None:
        raise RuntimeError("form solve failed")
    _SOLVE_CACHE[key] = best[1]
    return best[1]


def make_in_maps(x, weight, bias):
    av, bv, cq, cs = _solve_cached(np.asarray(weight, np.float64),
                                   np.asarray(bias, np.float64))
    shared = {
        "wm1": _lhs1(av, bv, GPB),
        "w2qe": _lhs2(cq, GPB, 0), "w2se": _lhs2(cs, GPB, 0),
        "w2qo": _lhs2(cq, GPB, 1), "w2so": _lhs2(cs, GPB, 1),
        "wm1t": _lhs1(av, bv, TAIL_GPB),
        "w2qt": _lhs2(cq, TAIL_GPB), "w2st": _lhs2(cs, TAIL_GPB),
    }
    x = np.ascontiguousarray(np.asarray(x, np.float16)).reshape(B, C, HW)
    return [dict(shared, xs=x[i*BPC:(i+1)*BPC]) for i in range(NCORES)]


def kernel(x, weight, bias, degree=3, **_unused):
    assert int(degree) == 3, "kernel specialized for degree=3"
    nc = build_nc(reps=1)
    in_maps = make_in_maps(x, weight, bias)
    res = run_bass_kernel_spmd(nc, in_maps, core_ids=list(range(NCORES)))
    out = np.empty((B, C, HW), np.float32)
    full_px = FULL_CHUNKS*GPB*ND
    for i in range(NCORES):
        ypv = np.asarray(res.results[i]["yp"])      # [2, 54, CK, 2048]
        ytv = np.asarray(res.results[i]["yt"])      # [48, 512]
        # yp[par, b*27+c*9+g, ck, k*512+j] -> y[b, c, ck*36864+g*4096+(2k+par)*512+j]
        ypr = ypv.reshape(2, BPC, C, GPB, FULL_CHUNKS, SPLIT//2, NCMP)
        yfull = ypr.transpose(1, 2, 4, 3, 5, 0, 6).reshape(BPC, C, full_px)
        ytr = ytv.reshape(BPC, C, TAIL_GPB, TAIL_ND).reshape(BPC, C, HW - full_px)
        core = np.concatenate([yfull, ytr], axis=2).astype(np.float32)
        out[i*BPC:(i+1)*BPC] = core
    return out.reshape(B, C, H, W)


if __name__ == "__main__":
    rng = np.random.default_rng(0)
    x = rng.uniform(0, 1, size=(B, C, H, W)).astype(np.float32)
    weight = rng.normal(size=(19, 3)).astype(np.float32)
    bias = rng.normal(size=(3,)).astype(np.float32)
    got = kernel(x, weight, bias, 3)
    print("ran; out shape", got.shape)


# revision 11
# speedup vs baseline: 1.9286x; 1.9286x over previous
"""Trainium2 Bass kernel for nn_ColorTransform: per-pixel degree-3 polynomial
color transform  y[b,c,h,w] = bias[c] + sum_f weight[f,c] * mono_f(x[b,:,h,w]).

Pure data parallel over batch across 8 cores (identical SPMD program).

Algorithm: represent the 3->19->3 degree-3 polynomial map as

    y_c = sum_{i<R} cq[i,c] * L_i^3 + cs[i,c] * L_i^2,   L_i = a_i.x + b_i

with R=7 forms solved per-(weight,bias) on the host by Gauss-Newton.  The
forms are cascade-rounded so (a_i, b_i) are exactly representable in f16
(the M1 matmul dtype); coefficients re-solved after each freeze.  Fallback
ladder R=7 -> 8 -> 9 -> 10 (R=10 always solvable by plain lstsq for any
target, quantization handled by the same cascade machinery).

On-chip, pixels are packed GROUPS-per-chunk x pixel-columns on the partition
dim (R form-rows per group, NG = BPC*GPB groups):
  PE  M1  -> P1 = wm1^T @ X  (block-diag forms)      [R*NG, NCMP] PSUM
  ACT     -> S = Square(P1)                          [R*NG] SBUF f32r
  DVE     -> Q = S * P1                              [R*NG] SBUF f32r
  PE  M2  -> P2 = w2q^T Q + w2s^T S  (PSUM accum)    [3*NG (+54 odd unit)]
  ACT/DVE -> pair copy-out (2 units share one PSUM tile: one copy per pair)
  DMA out f16 (host converts back to f32)
Full chunks: GPB=9 groups/batch x ND=4096 cols; tail chunk GPB=8 x 512.
"""
import numpy as np
from itertools import product as _product
from math import factorial as _factorial

import concourse.bass as bass
import concourse.tile as tile
from concourse import bacc, mybir
from concourse.bass_utils import run_bass_kernel_spmd

# ---------------------------------------------------------------- constants
B, C, H, W = 16, 3, 512, 512
HW = H * W
NCORES = 8
BPC = B // NCORES          # batches per core = 2

R = 7                      # affine forms per group (adaptive solve)
GPB = 9                    # groups per batch (full chunks)
ND = 4096                  # pixel columns per full chunk
NCMP = int(__import__("os").environ.get("K_NCMP", "512"))
SPLIT = ND // NCMP         # 4 units per chunk
FULL_CHUNKS = 7            # 9*4096*7 = 258048 px/plane
TAIL_GPB = 8               # tail: 8 groups/batch x 512 cols = 4096 px/plane
TAIL_ND = 512

import os
ACT_COPY_FRAC = float(os.environ.get("K_ACT_FRAC", "0.42"))
PS_BUFS = int(os.environ.get("K_PS_BUFS", "3"))
P2_BUFS = int(os.environ.get("K_P2_BUFS", "2"))
D2 = int(os.environ.get("K_D2", "1"))   # stage1 -> stage2 emission lag (units)
D3 = int(os.environ.get("K_D3", "2"))   # stage2 -> stage3 emission lag (units)
D4 = int(os.environ.get("K_D4", "1"))   # stage3 -> stage4 emission lag (pairs)
SQ_BUFS = int(os.environ.get("K_SQ_BUFS", "4"))
ABLATE = os.environ.get("K_ABLATE", "")
COPY_MODE = os.environ.get("K_COPY_MODE", "split")
UNROLL_REPS = int(os.environ.get("K_UNROLL_REPS", "4"))

assert GPB * ND * FULL_CHUNKS + TAIL_GPB * TAIL_ND == HW

# ------------------------------------------------------------ monomial alg
ALL_MONO = [(0,0,0),
    (1,0,0),(0,1,0),(0,0,1),
    (2,0,0),(1,1,0),(1,0,1),(0,2,0),(0,1,1),(0,0,2),
    (3,0,0),(2,1,0),(2,0,1),(1,2,0),(1,1,1),(1,0,2),(0,3,0),(0,2,1),(0,1,2),(0,0,3)]
MIDX = {m: i for i, m in enumerate(ALL_MONO)}

# multiply-by-x_k operators in the 20-dim monomial basis (degree<=2 source)
_XK = []
for k in range(3):
    m = np.zeros((20, 20))
    for mono, i in MIDX.items():
        if sum(mono) <= 2:
            up = list(mono); up[k] += 1
            m[MIDX[tuple(up)], i] = 1.0
    _XK.append(m)


def _expand_pow(a, b, power):
    out = np.zeros(20)
    for ks in _product(range(power + 1), repeat=4):
        if sum(ks) != power:
            continue
        k0, k1, k2, kb = ks
        mult = _factorial(power) / (_factorial(k0)*_factorial(k1)*_factorial(k2)*_factorial(kb))
        out[MIDX[(k0, k1, k2)]] += mult * a[0]**k0 * a[1]**k1 * a[2]**k2 * b**kb
    return out


def _basis(a, b, R_):
    e3 = np.stack([_expand_pow(a[i], b[i], 3) for i in range(R_)], axis=1)
    e2 = np.stack([_expand_pow(a[i], b[i], 2) for i in range(R_)], axis=1)
    e1 = np.stack([_expand_pow(a[i], b[i], 1) for i in range(R_)], axis=1)
    return e3, e2, e1


def _unpack(theta, R_):
    a = theta[:3*R_].reshape(R_, 3)
    b = theta[3*R_:4*R_]
    cq = theta[4*R_:7*R_].reshape(R_, 3)
    cs = theta[7*R_:10*R_].reshape(R_, 3)
    return a, b, cq, cs


def _pack(a, b, cq, cs):
    return np.concatenate([a.ravel(), b, cq.ravel(), cs.ravel()])


def _residual_jac(theta, R_, T, want_jac=True):
    a, b, cq, cs = _unpack(theta, R_)
    e3, e2, e1 = _basis(a, b, R_)
    M = e3 @ cq + e2 @ cs
    r = (M - T).ravel()            # row-major: 20 rows x 3 channels
    if not want_jac:
        return r, None
    J = np.zeros((60, 10*R_))
    for i in range(R_):
        # d/da_i[k]: 3*(Xk @ e2_i) * cq[i,c] + 2*(Xk @ e1_i) * cs[i,c]
        for k in range(3):
            col = 3*_XK[k] @ e2[:, i]
            col2 = 2*_XK[k] @ e1[:, i]
            for c in range(3):
                J[c::3, 3*i+k] += col * cq[i, c] + col2 * cs[i, c]
        # d/db_i: 3*e2_i*cq + 2*e1_i*cs
        for c in range(3):
            J[c::3, 3*R_+i] += 3*e2[:, i]*cq[i, c] + 2*e1[:, i]*cs[i, c]
        for c in range(3):
            J[c::3, 4*R_+3*i+c] = e3[:, i]
            J[c::3, 7*R_+3*i+c] = e2[:, i]
    return r, J


def _lm_solve(theta0, R_, T, free_mask, max_iter=200, tol=1e-13):
    """Levenberg-Marquardt over theta[free_mask]."""
    theta = theta0.copy()
    lam = 1e-3
    r, J = _residual_jac(theta, R_, T)
    cost = r @ r
    for _ in range(max_iter):
        Jf = J[:, free_mask]
        g = Jf.T @ r
        A = Jf.T @ Jf
        ok = False
        for _try in range(25):
            try:
                dx = np.linalg.solve(A + lam*np.diag(np.maximum(np.diag(A), 1e-10)), -g)
            except np.linalg.LinAlgError:
                lam *= 10; continue
            th_new = theta.copy()
            th_new[free_mask] = theta[free_mask] + dx
            r_new, _ = _residual_jac(th_new, R_, T, want_jac=False)
            c_new = r_new @ r_new
            if c_new < cost:
                theta, cost = th_new, c_new
                lam = max(lam*0.3, 1e-12)
                ok = True
                break
            lam *= 10
        if not ok:
            break
        r, J = _residual_jac(theta, R_, T)
        if cost < tol**2:
            break
    return theta, np.sqrt(cost)


def _coeff_lstsq(a, b, R_, T):
    e3, e2, _ = _basis(a, b, R_)
    A = np.concatenate([e3, e2], axis=1)
    Cf, *_ = np.linalg.lstsq(A, T, rcond=None)
    resid = np.linalg.norm(A @ Cf - T)
    return Cf[:R_], Cf[R_:], resid


def _normalize(theta, R_):
    a, b, cq, cs = _unpack(theta, R_)
    for i in range(R_):
        hi = b[i] + np.clip(a[i], 0, None).sum()
        lo = b[i] + np.clip(a[i], None, 0).sum()
        t = 1.0 / max(abs(hi), abs(lo), 1e-6)
        a[i] *= t; b[i] *= t
        cq[i] /= t**3; cs[i] /= t**2
    return _pack(a, b, cq, cs)


def _cascade_round(theta, R_, T):
    theta = theta.copy()
    frozen = np.zeros(10*R_, dtype=bool)
    for i in range(R_):
        idx = np.r_[3*i:3*i+3, 3*R_+i]
        theta[idx] = np.float64(np.float16(theta[idx]))
        frozen[idx] = True
        if i < R_ - 1:
            theta, r = _lm_solve(theta, R_, T, ~frozen, max_iter=60)
        else:
            a, b, cq, cs = _unpack(theta, R_)
            cq, cs, r = _coeff_lstsq(a, b, R_, T)
            theta = _pack(a, b, cq, cs)
    return theta, r


def _target(weight, bias):
    T = np.zeros((20, 3))
    T[0] = np.asarray(bias, np.float64)
    T[1:] = np.asarray(weight, np.float64)
    return T


def solve_forms(weight, bias, R_=R, n_init=8, seed=0):
    """-> (a [R,3] f16-exact, b [R] f16-exact, cq [R,3], cs [R,3], resid)."""
    T = _target(weight, bias)
    rg = np.random.default_rng(seed)
    best = None
    for trial in range(n_init):
        a0 = rg.normal(size=(R_, 3))
        a0 /= np.linalg.norm(a0, axis=1, keepdims=True)
        b0 = rg.normal(size=R_) * 0.5
        cq0, cs0, _ = _coeff_lstsq(a0, b0, R_, T)
        th0 = _pack(a0, b0, cq0, cs0)
        theta, r = _lm_solve(th0, R_, T, np.ones(10*R_, dtype=bool))
        if r > 1e-9:
            continue
        theta = _normalize(theta, R_)
        thq, rq = _cascade_round(theta, R_, T)
        a, b, cq, cs = _unpack(thq, R_)
        coefmag = max(np.abs(cq).max(), np.abs(cs).max())
        score = rq * (1 + 0.001*coefmag)
        if best is None or score < best[0]:
            best = (score, thq, rq)
        if rq < 5e-5 and coefmag < 50:
            break
    if best is None:
        return None
    a, b, cq, cs = _unpack(best[1], R_)
    return a, b, cq, cs, best[2]


# ---------------------------------------------------------- weight layouts
# X rows:  0 = ones; 1 + b*(3*gpb) + v*gpb + g
# P1 rows: b*(R*gpb) + i*gpb + g
# P2/O rows: b*(3*gpb) + c*gpb + g  (+ 3*gpb*BPC for odd units of a pair)

def _lhs1(av, bv, gpb):
    kx = 3*gpb*BPC + 1
    m = np.zeros((kx, R*gpb*BPC), np.float32)
    for b in range(BPC):
        for g in range(gpb):
            for i in range(R):
                col = b*R*gpb + i*gpb + g
                m[0, col] = bv[i]
                for v in range(C):
                    m[1 + b*3*gpb + v*gpb + g, col] = av[i, v]
    return m.astype(np.float16)


def _lhs2(coeff, gpb, parity=None):
    """M2 stationary: [R*gpb*BPC, OW] (parity=None) or zero-padded
    [R*gpb*BPC, 64+OW] with the coefficient block at column 64*parity."""
    ow = 3*gpb*BPC
    base = 0 if parity is None else 64*parity
    cols = ow if parity is None else 64 + ow if parity == 1 else 64 + ow
    if parity is not None:
        cols = 64 + ow
    m = np.zeros((R*gpb*BPC, cols if parity is not None else ow), np.float32)
    for b in range(BPC):
        for g in range(gpb):
            for i in range(R):
                for c in range(C):
                    m[b*R*gpb + i*gpb + g, base + b*3*gpb + c*gpb + g] = coeff[i, c]
    return m


# ---------------------------------------------------------------- bass build
_NC_CACHE = {}


def build_nc(reps=1, chunks=None):
    key = (reps, chunks)
    if key in _NC_CACHE:
        return _NC_CACHE[key]
    f32, f16, f32r = mybir.dt.float32, mybir.dt.float16, mybir.dt.float32r
    nc = bacc.Bacc("TRN2", target_bir_lowering=False, debug=False, num_devices=NCORES)

    KX = 3*GPB*BPC + 1          # 55
    RW = R*GPB*BPC              # 126
    OW = 3*GPB*BPC              # 54
    KXt = 3*TAIL_GPB*BPC + 1    # 49
    RWt = R*TAIL_GPB*BPC        # 112
    OWt = 3*TAIL_GPB*BPC       # 48

    xs = nc.dram_tensor("xs", [BPC, C, HW], f16, kind="ExternalInput")
    wm1 = nc.dram_tensor("wm1", [KX, RW], f16, kind="ExternalInput")
    w2qe = nc.dram_tensor("w2qe", [RW, 64 + OW], f32, kind="ExternalInput")
    w2se = nc.dram_tensor("w2se", [RW, 64 + OW], f32, kind="ExternalInput")
    w2qo = nc.dram_tensor("w2qo", [RW, 64 + OW], f32, kind="ExternalInput")
    w2so = nc.dram_tensor("w2so", [RW, 64 + OW], f32, kind="ExternalInput")
    wm1t = nc.dram_tensor("wm1t", [KXt, RWt], f16, kind="ExternalInput")
    w2qt = nc.dram_tensor("w2qt", [RWt, OWt], f32, kind="ExternalInput")
    w2st = nc.dram_tensor("w2st", [RWt, OWt], f32, kind="ExternalInput")
    yp = nc.dram_tensor("yp", [2, OW, FULL_CHUNKS if chunks is None else max(chunks, 1), ND//2],
                        f16, kind="ExternalOutput")
    yt = nc.dram_tensor("yt", [OWt, TAIL_ND], f16, kind="ExternalOutput")

    asp = (int(NCMP * ACT_COPY_FRAC) // 2) * 2

    with tile.TileContext(nc) as tc:
        with (
            tc.tile_pool(name="wpool", bufs=1) as wpool,
            tc.tile_pool(name="xpool", bufs=int(os.environ.get("K_X_BUFS", "4"))) as xpool,
            tc.tile_pool(name="spool", bufs=SQ_BUFS) as spool,
            tc.tile_pool(name="qpool", bufs=SQ_BUFS) as qpool,
            tc.tile_pool(name="opool", bufs=int(os.environ.get("K_O_BUFS", "3"))) as opool,
            tc.tile_pool(name="p1pool", bufs=PS_BUFS, space="PSUM") as p1pool,
            tc.tile_pool(name="p2pool", bufs=P2_BUFS, space="PSUM") as p2pool,
        ):
            def load_w(name, dram, shape, dt_, round_to=None):
                t = wpool.tile(shape, dt_, tag=name)
                nc.sync.dma_start(t[:], dram[:])
                if round_to is None:
                    return t
                tr = wpool.tile(shape, round_to, tag=name + "r")
                nc.vector.tensor_copy(tr[:], t[:])
                return tr

            wm1_sb = load_w("wm1", wm1, [KX, RW], f16)
            w2qe_r = load_w("w2qe", w2qe, [RW, 64 + OW], f32, f32r)
            w2se_r = load_w("w2se", w2se, [RW, 64 + OW], f32, f32r)
            w2qo_r = load_w("w2qo", w2qo, [RW, 64 + OW], f32, f32r)
            w2so_r = load_w("w2so", w2so, [RW, 64 + OW], f32, f32r)
            wm1t_sb = load_w("wm1t", wm1t, [KXt, RWt], f16)
            w2qt_r = load_w("w2qt", w2qt, [RWt, OWt], f32, f32r)
            w2st_r = load_w("w2st", w2st, [RWt, OWt], f32, f32r)

            for _ in range(int(os.environ.get("K_X_BUFS", "4"))):
                xt0 = xpool.tile([KX, ND], f16, tag="X")
                nc.gpsimd.memset(xt0[0:1, :], 1.0)
            if ABLATE == "nosq":
                for _ in range(SQ_BUFS):
                    s0 = spool.tile([RW, 2*NCMP], f32r, tag="S")
                    nc.gpsimd.memset(s0[:], 0.25)
                    q0 = qpool.tile([RW, 2*NCMP], f32r, tag="Q")
                    nc.gpsimd.memset(q0[:], 0.125)

            # ---------------- stages -------------------------------------
            def stage0(ck, lo, gpb, nd, st):
                """Prefetch chunk ck's X tile (DMA-in)."""
                kx = 3*gpb*BPC + 1
                xt = xpool.tile([kx, nd], f16, tag="X",
                                name=f"xt{'T' if gpb == TAIL_GPB else ''}")
                if gpb == TAIL_GPB:
                    nc.gpsimd.memset(xt[0:1, :], 1.0)
                nc.sync.dma_start(
                    xt[1:1+2*3*gpb],
                    xs[:, :, lo:lo+gpb*nd].rearrange(
                        "b v (g n) -> (b v) g n", n=nd))
                st[ck] = xt

            # A "pair" = 2 consecutive NCMP-column units sharing one P1 tile
            # (M1 writes halves; square/cube run once over both halves).
            # A "quad" = 2 consecutive pairs sharing one P2 tile
            # (even/odd unit -> partition base 0/64, pair parity -> col half);
            # one copy-out per quad.
            def stage1(pr, st):
                """M1 for both halves of pair -> P1 [rw, 2*ncmp]."""
                ck, lo, gpb, nd, ncmp, p, wset, nun = pr
                rw = R*gpb*BPC
                xt = st[ck]
                p1 = p1pool.tile([rw, 2*NCMP], f32, tag="P1")
                for h in range(nun):
                    cl = (2*p + h)*ncmp
                    nc.tensor.matmul(p1[:, h*ncmp:(h+1)*ncmp], wset[0][:],
                                     xt[:, cl:cl+ncmp], start=True, stop=True)
                return p1

            def stage2(pr, p1):
                ck, lo, gpb, nd, ncmp, p, wset, nun = pr
                rw = R*gpb*BPC
                w = nun*ncmp
                s = spool.tile([rw, 2*NCMP], f32r, tag="S")
                if ABLATE != "nosq":
                    nc.scalar.square(s[:, :w], p1[:, :w])
                qq = qpool.tile([rw, 2*NCMP], f32r, tag="Q")
                if ABLATE != "nosq":
                    nc.vector.tensor_mul(qq[:, :w], s[:, :w], p1[:, :w])
                return s, qq

            def stage3(pr, s, qq, st):
                """M2 into row-paired PSUM [64+OW, NCMP] via zero-padded
                block lhsT (both units accumulate, dst base 0)."""
                ck, lo, gpb, nd, ncmp, p, wset, nun = pr
                ow = 3*gpb*BPC
                tail = gpb == TAIL_GPB
                rows = ow if tail else 64 + ow
                p2 = p2pool.tile([rows, NCMP], f32, tag="P2",
                                 name="p2T" if tail else "p2")
                for h in range(nun):
                    cb = h*ncmp
                    qw = wset[1] if tail else wset[1][h]
                    sw = wset[2] if tail else wset[2][h]
                    nc.tensor.matmul(p2[:, :ncmp], qw[:], qq[:, cb:cb+ncmp],
                                     start=(h == 0), stop=False)
                    nc.tensor.matmul(p2[:, :ncmp], sw[:], s[:, cb:cb+ncmp],
                                     start=False, stop=(h == nun - 1))
                if tail:
                    st["o"] = opool.tile([ow, nd], f16, tag="O", name="oT")
                elif p == 0:
                    st["o"] = opool.tile([64 + ow, nd//2], f16, tag="O",
                                         name="o")
                return (pr, p2, st["o"])

            def stage4(pair):
                """Copy pair PSUM -> O (ACT/DVE col split); DMA at chunk end."""
                if ABLATE == "nocopy":
                    return
                pr, p2, o = pair
                ck, lo, gpb, nd, ncmp, p, wset, nun = pr
                ow = 3*gpb*BPC
                tail = gpb == TAIL_GPB
                if tail:
                    a = (int(ncmp*ACT_COPY_FRAC)//2)*2
                    nc.scalar.copy(o[:, :a], p2[:, :a])
                    nc.vector.tensor_copy(o[:, a:ncmp], p2[:, a:ncmp])
                    if ABLATE != "nodma":
                        nc.sync.dma_start(yt[:], o[:])
                    return
                cl = p*NCMP
                if COPY_MODE == "alt":
                    if p % 2 == 0:
                        nc.scalar.copy(o[:, cl:cl+NCMP], p2[:, :NCMP])
                    else:
                        nc.vector.tensor_copy(o[:, cl:cl+NCMP], p2[:, :NCMP])
                else:
                    nc.scalar.copy(o[:, cl:cl+asp], p2[:, :asp])
                    nc.vector.tensor_copy(o[:, cl+asp:cl+NCMP], p2[:, asp:NCMP])
                if (2*p + nun)*ncmp == nd and ABLATE != "nodma":
                    # device-order output: host inverse-permutes (free here)
                    for pp in range(2):
                        nc.sync.dma_start(yp[pp, :, ck, :],
                                          o[pp*64:pp*64+ow])

            def body():
                nfull = FULL_CHUNKS if chunks is None else chunks
                pairs = [(k, k*GPB*ND, GPB, ND, NCMP, p,
                          (wm1_sb, (w2qe_r, w2qo_r), (w2se_r, w2so_r)), 2)
                         for k in range(nfull) for p in range(SPLIT//2)]
                if chunks is None:
                    pairs.append((nfull, FULL_CHUNKS*GPB*ND, TAIL_GPB, TAIL_ND,
                                  TAIL_ND, 0, (wm1t_sb, w2qt_r, w2st_r), 1))
                chunk_info = {}
                for pr in pairs:
                    chunk_info.setdefault(pr[0], (pr[1], pr[2], pr[3]))
                st1, st3 = {}, {}
                q2, q3, q4 = [], [], []

                def pump(force=False):
                    if q2 and (force or len(q2) > D2 - 1):
                        pr, p1 = q2.pop(0)
                        s, qq = stage2(pr, p1)
                        q3.append((pr, s, qq))
                    if q3 and (force or len(q3) > D3 - 1):
                        pr, s, qq = q3.pop(0)
                        quad = stage3(pr, s, qq, st3)
                        if quad is not None:
                            q4.append(quad)
                    if q4 and (force or len(q4) > D4 - 1):
                        stage4(q4.pop(0))

                stage0(0, *chunk_info[0], st1)
                for pr in pairs:
                    ck, lo, gpb, nd, ncmp, p, wset, nun = pr
                    if p == 0 and ck + 1 in chunk_info:
                        stage0(ck + 1, *chunk_info[ck + 1], st1)
                    p1 = stage1(pr, st1)
                    q2.append((pr, p1))
                    pump()
                while q2 or q3 or q4:
                    pump(force=True)

            if reps == 1:
                for _u in range(int(os.environ.get("K_UNROLL", "1"))):
                    body()
            else:
                hint = (mybir.EngineType.PE, mybir.EngineType.Activation,
                        mybir.EngineType.DVE, mybir.EngineType.SP)
                u = UNROLL_REPS
                while u > 1 and reps % u:
                    u -= 1
                with tc.For_i(0, reps // u, 1, hint_engines=hint):
                    for _ in range(u):
                        body()

    nc.compile()
    _NC_CACHE[key] = nc
    return nc


# ------------------------------------------------------------- host wrapper
_SOLVE_CACHE = {}


def _eval_maxrel(a, b, cq, cs, weight, bias, n=60000):
    """Quick numpy check of the decomposition through the f16/f32 pipeline."""
    rng = np.random.default_rng(11)
    x = rng.uniform(0, 1, size=(n, 3))
    F = np.stack([x[:, 0]**m[0] * x[:, 1]**m[1] * x[:, 2]**m[2]
                  for m in ALL_MONO], axis=1)
    y_ref = F @ _target(weight, bias)
    L = x.astype(np.float16).astype(np.float32) @ a.T.astype(np.float32) \
        + b.astype(np.float32)
    S = L*L
    Q = S*L
    yy = (Q @ cq.astype(np.float32) + S @ cs.astype(np.float32))
    yy = yy.astype(np.float16).astype(np.float32)
    return np.abs(yy - y_ref).max() / max(np.abs(y_ref).max(), 1e-9)


def _solve_cached(weight, bias):
    key = (weight.tobytes(), bias.tobytes())
    if key in _SOLVE_CACHE:
        return _SOLVE_CACHE[key]
    best = None
    for seed in (0, 1, 2, 3):
        out = solve_forms(weight, bias, R_=R, seed=seed)
        if out is None:
            continue
        a, b, cq, cs, _resid = out
        err = _eval_maxrel(a, b, cq, cs, weight, bias)
        if best is None or err < best[0]:
            best = (err, (a, b, cq, cs))
        if err < 5e-3:
            break
    if best is None:
        raise RuntimeError("form solve failed")
    _SOLVE_CACHE[key] = best[1]
    return best[1]


def make_in_maps(x, weight, bias):
    av, bv, cq, cs = _solve_cached(np.asarray(weight, np.float64),
                                   np.asarray(bias, np.float64))
    shared = {
        "wm1": _lhs1(av, bv, GPB),
        "w2qe": _lhs2(cq, GPB, 0), "w2se": _lhs2(cs, GPB, 0),
        "w2qo": _lhs2(cq, GPB, 1), "w2so": _lhs2(cs, GPB, 1),
        "wm1t": _lhs1(av, bv, TAIL_GPB),
        "w2qt": _lhs2(cq, TAIL_GPB), "w2st": _lhs2(cs, TAIL_GPB),
    }
    x = np.ascontiguousarray(np.asarray(x, np.float16)).reshape(B, C, HW)
    return [dict(shared, xs=x[i*BPC:(i+1)*BPC]) for i in range(NCORES)]


def kernel(x, weight, bias, degree=3, **_unused):
    assert int(degree) == 3, "kernel specialized for degree=3"
    nc = build_nc(reps=1)
    in_maps = make_in_maps(x, weight, bias)
    res = run_bass_kernel_spmd(nc, in_maps, core_ids=list(range(NCORES)))
    out = np.empty((B, C, HW), np.float32)
    full_px = FULL_CHUNKS*GPB*ND
    for i in range(NCORES):
        ypv = np.asarray(res.results[i]["yp"])      # [2, 54, CK, 2048]
        ytv = np.asarray(res.results[i]["yt"])      # [48, 512]
        # yp[par, b*27+c*9+g, ck, k*512+j] -> y[b, c, ck*36864+g*4096+(2k+par)*512+j]
        ypr = ypv.reshape(2, BPC, C, GPB, FULL_CHUNKS, SPLIT//2, NCMP)
        yfull = ypr.transpose(1, 2, 4, 3, 5, 0, 6).reshape(BPC, C, full_px)
        ytr = ytv.reshape(BPC, C, TAIL_GPB, TAIL_ND).reshape(BPC, C, HW - full_px)
        core = np.concatenate([yfull, ytr], axis=2).astype(np.float32)
        out[i*BPC:(i+1)*BPC] = core
    return out.reshape(B, C, H, W)


if __name__ == "__main__":
    rng = np.random.default_rng(0)
    x = rng.uniform(0, 1, size=(B, C, H, W)).astype(np.float32)
    weight = rng.normal(size=(19, 3)).astype(np.float32)
    bias = rng.normal(size=(3,)).astype(np.float32)
    got = kernel(x, weight, bias, 3)
    print("ran; out shape", got.shape)
